# revision 12
# baseline (speedup 1.0000x reference)
"""Trainium2 Bass kernel for nn_DecoderTopDown (top-down attention LSTM decoder).

Strategy (8 NeuronCores, tensor-parallel over gate/hidden dims):
  - Each core owns a 128-wide slice of H1 and H2 (gates reordered [i|f|o|g]).
  - Per step: gate GEMMs in [M=batch, N=gates] layout (batch on PSUM partition),
    recurrent h exchanged via small AllGathers of transposed bf16 slices.
  - Attention: R=36 regions split across cores (5/5/5/5/4/4/4/4, padded to 5);
    logits exchanged via a tiny AllGather; softmax replicated; the atten-weighted
    sum over regions is computed on the PE as block-diagonal matmuls against a
    precomputed VW2 = Vmat @ Wi2[:, :V].T (per-core gate slice), accumulated
    straight into the LSTM2 gate PSUM.
  - Step-invariant input terms precomputed: pre1[t] = emb_t@Wie.T + uv@Wiv.T + b1.
  - Vocab projection (Wl) done after the loop as one big GEMM, vocab-sharded,
    over ragged-compacted h2 history (only active (b,t) pairs — lengths baked).
  - sigmoid(x) = (1+tanh(x/2))/2 so the whole kernel stays on one ACT table set.

kernel(**inputs) takes FULL inputs, returns FULL [B, T, VOC] float32 output.
"""
import sys, os
sys.path.insert(0, "/opt/trn_rl_repo")

import numpy as np
import ml_dtypes

BF16 = ml_dtypes.bfloat16

# Problem dims (hardcoded per contest rules)
B, R, T = 128, 36, 40
E, V, H1, H2, PH, VOC = 1024, 2048, 1024, 1024, 256, 10000
NC_ = 8                    # cores
GS = 4 * H1 // NC_         # per-core gate slice = 512
HS = H1 // NC_             # per-core hidden slice = 128
RP = 5                     # padded r's per core
RT = NC_ * RP              # padded total r rows = 40
VS = VOC // NC_            # vocab slice = 1250
KT1 = H1 // 128            # 8 k-tiles for H-sized contractions
KTV = V // 128             # 16 k-tiles for V-sized contractions
NPAIR = B // 2             # 64 block-diag pairs (blocks at partitions 0 and 64)
BDK = 64 + RT              # 104: rows [0,40) = even b, [64,104) = odd b

_cache = {}
LAST_NC = None
LAST_IN_MAPS = None


def _r_assign():
    """r-split across cores: cores 0-3 get 5, cores 4-7 get 4 (+1 pad).
    Returns per-core lists of global r (or -1 for pad) and the permuted row->r map."""
    per_core = []
    row_r = []
    nxt = 0
    for c in range(NC_):
        cnt = 5 if c < 4 else 4
        rs = list(range(nxt, nxt + cnt)) + [-1] * (RP - cnt)
        nxt += cnt
        per_core.append(rs)
        row_r += rs
    assert nxt == R
    return per_core, row_r   # row_r: length 40, -1 = pad


def _prep_inputs(inputs):
    """Host-side prep: gather embeddings, transpose/slice/cast weights per core."""
    Vmat = inputs["Vmat"].astype(np.float32)
    uv = inputs["union_vfeats"].astype(np.float32)
    captions = inputs["captions"]
    lengths = np.asarray(inputs["lengths"]).astype(np.int64)
    embW = inputs["embed_W"].astype(np.float32)
    Wi1, Wh1, b1 = inputs["Wi1"], inputs["Wh1"], inputs["b1"]
    Wi2, Wh2, b2 = inputs["Wi2"], inputs["Wh2"], inputs["b2"]
    Wva, bva = inputs["Wva"], inputs["bva"]
    Wha, bha = inputs["Wha"], inputs["bha"]
    wa = inputs["wa"]
    Wl, bl = inputs["Wl"], inputs["bl"]

    per_core_r, row_r = _r_assign()

    # active counts per step (lengths sorted descending)
    n_t = [int((lengths > t).sum()) for t in range(T)]
    offs = np.concatenate([[0], np.cumsum(n_t)]).astype(np.int64)

    emb = embW[captions]                              # [B,T,E]
    embT = np.ascontiguousarray(emb.transpose(1, 2, 0)).astype(BF16)   # [T,E,B]
    uvT = np.ascontiguousarray(uv.T).astype(BF16)     # [V,B]
    VmatT = np.ascontiguousarray(Vmat.transpose(2, 0, 1))  # [V,B,R] fp32

    WhaT = np.ascontiguousarray(Wha.T).astype(BF16)   # [H1,PH]
    bhab = np.broadcast_to(bha[None, :], (B, PH)).astype(np.float32).copy()
    wab = np.broadcast_to(wa[None, :], (B, PH)).astype(BF16).copy()
    bvab = np.broadcast_to(bva[None, :], (B, PH)).astype(np.float32).copy()

    # VmatT3: block-diag einsum rhs source [V, NPAIR, 104] (zeros at pad-r rows)
    VmatT3 = np.zeros((V, NPAIR, BDK), dtype=np.float32)
    for c2 in range(2):
        bs = 2 * np.arange(NPAIR) + c2
        for j, r in enumerate(row_r):
            if r >= 0:
                VmatT3[:, :, 64 * c2 + j] = VmatT[:, bs, r]
    VmatT3 = VmatT3.astype(BF16)

    in_maps = []
    for c in range(NC_):
        hs = slice(HS * c, HS * (c + 1))
        # gate rows, reordered [i|f|o|g]
        def perm(Hn):
            base = np.arange(HS * c, HS * (c + 1))
            return np.concatenate([base, base + Hn, base + 3 * Hn, base + 2 * Hn])
        p1 = perm(H1); p2 = perm(H2)

        W1h2T = np.ascontiguousarray(Wi1[p1, 0:H2].T).astype(BF16)        # [1024,512]
        Wh1T = np.ascontiguousarray(Wh1[p1, :].T).astype(BF16)            # [1024,512]
        W1eT = np.ascontiguousarray(Wi1[p1, H2 + V:].T).astype(BF16)      # [1024,512]
        W1vT = np.ascontiguousarray(Wi1[p1, H2:H2 + V].T).astype(BF16)    # [2048,512]
        W2h1T = np.ascontiguousarray(Wi2[p2, V:].T).astype(BF16)          # [1024,512]
        Wh2T = np.ascontiguousarray(Wh2[p2, :].T).astype(BF16)            # [1024,512]
        W2vT = np.ascontiguousarray(Wi2[p2, 0:V].T).astype(BF16)          # [2048,512]
        b1b = np.broadcast_to(b1[p1][None, :], (B, GS)).astype(np.float32).copy()
        b2b = np.broadcast_to(b2[p2][None, :], (B, GS)).astype(np.float32).copy()

        vs = slice(VS * c, VS * (c + 1))
        WlT = np.ascontiguousarray(Wl[vs, :].T).astype(BF16)              # [1024,1250]
        blb = np.broadcast_to(bl[vs][None, :], (B, VS)).astype(np.float32).copy()

        # attention r-slice: VmatTr [V, RP, B], zeros for pad
        rs = per_core_r[c]
        VmatTr = np.zeros((V, RP, B), dtype=np.float32)
        for j, r in enumerate(rs):
            if r >= 0:
                VmatTr[:, j, :] = VmatT[:, :, r]
        VmatTr = VmatTr.astype(BF16)
        lmask = np.array([[1.0] if r >= 0 else [0.0] for r in rs], np.float32)
        loff = np.array([[0.0] if r >= 0 else [-30.0] for r in rs], np.float32)

        in_maps.append(dict(
            embT=embT, uvT=uvT, VmatT3=VmatT3, VmatTr=VmatTr,
            W1h2T=W1h2T, Wh1T=Wh1T, W1eT=W1eT, W1vT=W1vT,
            W2h1T=W2h1T, Wh2T=Wh2T, W2vT=W2vT,
            WhaT=WhaT, bhab=bhab, wab=wab, bvab=bvab,
            WlT=WlT, blb=blb, b1b=b1b, b2b=b2b,
            lmask=lmask, loff=loff,
        ))
    return in_maps, n_t, offs


def _build(n_t, offs, Tsteps):
    """Build + compile the Bass program (lengths-specialized)."""
    from concourse import bass, bacc, tile, mybir, masks
    NO_DMAT = bool(int(os.environ.get("K_NO_DMAT", "0")))
    NO_CC = bool(int(os.environ.get("K_NO_CC", "0")))  # debug: wrong results, no collectives
    PHASE = os.environ.get("K_PHASE", "all")  # pre | loop | all

    f32 = mybir.dt.float32
    bf16 = mybir.dt.bfloat16
    AT = mybir.ActivationFunctionType
    OP = mybir.AluOpType
    NCOMP = int(offs[Tsteps])           # total compacted (b,t) columns
    NMT = (NCOMP + 127) // 128          # compacted M-tiles for Wl

    nc = bacc.Bacc("TRN2", target_bir_lowering=False, debug=False, num_devices=NC_)

    # ---------------- I/O declarations ----------------
    def din(name, shape, dt=bf16):
        return nc.dram_tensor(name, shape, dt, kind="ExternalInput")

    embT = din("embT", [T, E, B])
    uvT = din("uvT", [V, B])
    VmatT3 = din("VmatT3", [V, NPAIR, BDK])
    VmatTr = din("VmatTr", [V, RP, B])
    W1h2T = din("W1h2T", [H2, GS]); Wh1T = din("Wh1T", [H1, GS])
    W1eT = din("W1eT", [E, GS]); W1vT = din("W1vT", [V, GS])
    W2h1T = din("W2h1T", [H1, GS]); Wh2T = din("Wh2T", [H2, GS])
    W2vT = din("W2vT", [V, GS])
    WhaT = din("WhaT", [H1, PH])
    bhab = din("bhab", [B, PH], f32)
    wab = din("wab", [B, PH])
    bvab = din("bvab", [B, PH], f32)
    WlT = din("WlT", [H2, VS])
    blb = din("blb", [B, VS], f32)
    b1b = din("b1b", [B, GS], f32); b2b = din("b2b", [B, GS], f32)
    lmask = din("lmask", [RP, 1], f32); loff = din("loff", [RP, 1], f32)

    out = nc.dram_tensor("out", [B, T, VS], f32, kind="ExternalOutput")

    RG = [list(range(NC_))]

    with tile.TileContext(nc) as tc:
      with (
        tc.tile_pool(name="persist", bufs=1) as P,
        tc.tile_pool(name="loopres", bufs=1) as LP,
        tc.tile_pool(name="dram", bufs=1, space="DRAM") as D,
        tc.tile_pool(name="bounce", bufs=3, space="DRAM") as BP,
      ):
        # ---------- persistent SBUF state ----------
        ident = P.tile([128, 128], bf16)
        masks.make_identity(nc, ident[:])
        identf = P.tile([128, 128], f32)
        masks.make_identity(nc, identf[:])

        h1T = P.tile([128, KT1, B], bf16)     # gathered h1^T  [feat, b] tiles
        h2T = P.tile([128, KT1, B], bf16)
        h1n = P.tile([B, HS], bf16)           # own slice, [b, feat]
        h2n = P.tile([B, HS], bf16)
        c1 = P.tile([B, HS], f32)
        c2 = P.tile([B, HS], f32)
        for tbuf in (h1T, h2T, h1n, h2n, c1, c2):
            nc.vector.memset(tbuf[:], 0.0)

        atten_bd = LP.tile([128, NPAIR * 128], bf16)  # block-diag lhsT (zeros persist)
        nc.vector.memset(atten_bd[:], 0.0)

        VW2 = LP.tile([128, NPAIR, GS], bf16)   # einsum rhs tiles (BDK=104 rows used)
        Va = LP.tile([B, RP, PH], bf16)        # attention bias term, r-sliced

        # loop-resident weights
        w1h2 = LP.tile([128, KT1, GS], bf16)
        wh1 = LP.tile([128, KT1, GS], bf16)
        w2h1 = LP.tile([128, KT1, GS], bf16)
        wh2 = LP.tile([128, KT1, GS], bf16)
        wha = LP.tile([128, KT1, PH], bf16)
        for dst, s_ in ((w1h2, W1h2T), (wh1, Wh1T), (w2h1, W2h1T),
                        (wh2, Wh2T), (wha, WhaT)):
            nc.sync.dma_start(dst[:], s_.ap().rearrange("(k p) n -> p k n", p=128))

        b2s = P.tile([B, GS], f32); nc.sync.dma_start(b2s[:], b2b[:])
        bhas = P.tile([B, PH], f32); nc.sync.dma_start(bhas[:], bhab[:])
        was = P.tile([B, PH], bf16); nc.sync.dma_start(was[:], wab[:])
        lmasks = P.tile([RP, 1], f32); nc.sync.dma_start(lmasks[:], lmask[:])
        loffs = P.tile([RP, 1], f32); nc.sync.dma_start(loffs[:], loff[:])

        # DRAM intermediates
        pre1_d = D.tile([T, B, GS], bf16)
        h2a_d = D.tile([H2, max(NCOMP, 1)], bf16)

        # ================= PRECOMPUTE =================
        with (
            tc.tile_pool(name="pre_sb", bufs=3) as PS,
            tc.tile_pool(name="pre_w0", bufs=1) as PW0,
            tc.tile_pool(name="pre_ps", bufs=2, space="PSUM") as PP,
            tc.tile_pool(name="pre_psg", bufs=1, space="PSUM") as PPG,
        ):
            base1 = PW0.tile([B, GS], f32)
            # --- base1 = uv @ W1v.T + b1 ---
            with tc.tile_pool(name="pre_w1", bufs=1) as PW:
                b1s = PW.tile([B, GS], f32); nc.sync.dma_start(b1s[:], b1b[:])
                w1v = PW.tile([128, KTV, GS], bf16)
                nc.sync.dma_start(w1v[:], W1vT.ap().rearrange("(k p) n -> p k n", p=128))
                uvs = PW.tile([128, KTV, B], bf16)
                nc.sync.dma_start(uvs[:], uvT.ap().rearrange("(k p) n -> p k n", p=128))
                ps0 = PP.tile([B, GS], f32, tag="pre")
                for k in range(KTV):
                    nc.tensor.matmul(ps0[:], uvs[:, k, :], w1v[:, k, :],
                                     start=(k == 0), stop=(k == KTV - 1))
                nc.vector.tensor_tensor(out=base1[:], in0=ps0[:], in1=b1s[:], op=OP.add)

            # --- pre1[t] = embT[t] @ W1e.T + base1 -> DRAM bf16 ---
            with tc.tile_pool(name="pre_w2", bufs=1) as PW:
                w1e = PW.tile([128, KT1, GS], bf16)
                nc.sync.dma_start(w1e[:], W1eT.ap().rearrange("(k p) n -> p k n", p=128))
                for t in range(Tsteps):
                    et = PS.tile([128, KT1, B], bf16, tag="et")
                    nc.sync.dma_start(
                        et[:], embT.ap()[t].rearrange("(k p) n -> p k n", p=128))
                    pps = PP.tile([B, GS], f32, tag="pre")
                    for k in range(KT1):
                        nc.tensor.matmul(pps[:], et[:, k, :], w1e[:, k, :],
                                         start=(k == 0), stop=(k == KT1 - 1))
                    pre_t = PS.tile([B, GS], bf16, tag="pre_t")
                    nc.vector.tensor_tensor(out=pre_t[:], in0=pps[:], in1=base1[:], op=OP.add)
                    nc.sync.dma_start(pre1_d[:][t], pre_t[:])

            # --- Va[b, j, ph] = (Vmat @ Wva.T + bva) for own r's ---
            with tc.tile_pool(name="pre_w3", bufs=1) as PW:
                bvas = PW.tile([B, PH], f32); nc.sync.dma_start(bvas[:], bvab[:])
                wva = PW.tile([128, KTV, PH], bf16)
                nc.sync.dma_start(
                    wva[:], nc.dram_tensor("WvaT", [V, PH], bf16, kind="ExternalInput")
                    .ap().rearrange("(k p) n -> p k n", p=128))
                for j in range(RP):
                    vps = PP.tile([B, GS], f32, tag="pre")
                    for k in range(KTV):
                        vtr = PS.tile([128, B], bf16, tag="vtr")
                        nc.sync.dma_start(
                            vtr[:], VmatTr.ap().rearrange("(k p) j n -> k p j n", p=128)[k, :, j])
                        nc.tensor.matmul(vps[:, 0:PH], vtr[:], wva[:, k, :],
                                         start=(k == 0), stop=(k == KTV - 1))
                    nc.vector.tensor_tensor(out=Va[:, j, :], in0=vps[:, 0:PH], in1=bvas[:], op=OP.add)

            # --- VW2 pair tiles: VmatT3 @ W2v.T -> [104, GS] bf16 each ---
            with tc.tile_pool(name="pre_w4", bufs=1) as PW:
                w2v = PW.tile([128, KTV, GS], bf16)
                nc.sync.dma_start(w2v[:], W2vT.ap().rearrange("(k p) n -> p k n", p=128))
                GRP = 6
                for i0 in range(0, NPAIR, GRP):
                    cnt = min(GRP, NPAIR - i0)
                    pss = [PPG.tile([BDK, GS], f32, tag=f"vw{g}", name=f"vw{g}") for g in range(cnt)]
                    for k in range(KTV):
                        v3 = PS.tile([128, GRP * BDK], bf16, tag="v3")
                        nc.sync.dma_start(
                            v3[:, 0:cnt * BDK],
                            VmatT3.ap().rearrange("(k p) i n -> k p i n", p=128)[k, :, i0:i0 + cnt]
                        )
                        for g in range(cnt):
                            nc.tensor.matmul(pss[g][:], v3[:, g * BDK:(g + 1) * BDK],
                                             w2v[:, k, :], start=(k == 0), stop=(k == KTV - 1))
                    for g in range(cnt):
                        nc.scalar.copy(VW2[0:BDK, i0 + g, :], pss[g][:])

        # ================= RECURRENT LOOP =================
        if PHASE == "pre":
            Tsteps = 0
        with (
            tc.tile_pool(name="work", bufs=2) as W,
            tc.tile_pool(name="pg1", bufs=2, space="PSUM") as PG1,
            tc.tile_pool(name="pg2", bufs=2, space="PSUM") as PG2,
            tc.tile_pool(name="psm", bufs=2, space="PSUM") as PSM,
        ):
            for t in range(Tsteps):
                n = n_t[t]
                # ---- g1 = W1h2@h2 + Wh1@h1 (+pre1) ----
                g1ps = PG1.tile([B, GS], f32, tag="g1")
                for k in range(KT1):
                    nc.tensor.matmul(g1ps[0:n, :], h2T[:, k, 0:n], w1h2[:, k, :],
                                     start=(k == 0), stop=False)
                for k in range(KT1):
                    nc.tensor.matmul(g1ps[0:n, :], h1T[:, k, 0:n], wh1[:, k, :],
                                     start=False, stop=(k == KT1 - 1))
                pre_t = W.tile([B, GS], bf16, tag="pre_t")
                nc.sync.dma_start(pre_t[:], pre1_d[:][t])
                gs1 = W.tile([B, GS], f32, tag="gs1")
                nc.vector.tensor_tensor(out=gs1[0:n, :], in0=g1ps[0:n, :],
                                        in1=pre_t[0:n, :], op=OP.add)
                # ---- LSTM1 cell ([i|f|o|g]; sigmoid via tanh) ----
                tio1 = W.tile([B, 3 * HS], f32, tag="tio1")
                nc.scalar.activation(tio1[0:n, :], gs1[0:n, 0:3 * HS], AT.Tanh, scale=0.5)
                tg1 = W.tile([B, HS], f32, tag="tg1")
                nc.scalar.activation(tg1[0:n, :], gs1[0:n, 3 * HS:], AT.Tanh)
                sg1 = W.tile([B, 3 * HS], f32, tag="sg1")
                nc.vector.tensor_scalar(out=sg1[0:n, :], in0=tio1[0:n, :],
                                        scalar1=0.5, scalar2=0.5, op0=OP.mult, op1=OP.add)
                aa = W.tile([B, HS], f32, tag="aa")
                nc.vector.tensor_tensor(out=aa[0:n, :], in0=sg1[0:n, HS:2 * HS],
                                        in1=c1[0:n, :], op=OP.mult)
                bb = W.tile([B, HS], f32, tag="bb")
                nc.vector.tensor_tensor(out=bb[0:n, :], in0=sg1[0:n, 0:HS],
                                        in1=tg1[0:n, :], op=OP.mult)
                nc.vector.tensor_tensor(out=c1[0:n, :], in0=aa[0:n, :],
                                        in1=bb[0:n, :], op=OP.add)
                tc1 = W.tile([B, HS], f32, tag="tc1")
                nc.scalar.activation(tc1[0:n, :], c1[0:n, :], AT.Tanh)
                nc.vector.tensor_tensor(out=h1n[0:n, :], in0=sg1[0:n, 2 * HS:],
                                        in1=tc1[0:n, :], op=OP.mult)

                # ---- AllGather h1T ----
                h1loc = W.tile([128, B], bf16, tag="h1loc")
                if NO_DMAT:
                    tp1 = PSM.tile([128, B], bf16, tag="ltp", name="tp1")
                    nc.tensor.transpose(tp1[:], h1n[:], ident[:])
                    nc.vector.tensor_copy(h1loc[:], tp1[:])
                else:
                    nc.sync.dma_start_transpose(h1loc[:], h1n[:])
                if NO_CC:
                    for kk in range(KT1):
                        nc.vector.tensor_copy(h1T[:, kk, :], h1loc[:])
                else:
                    agi1 = BP.tile([128, B], bf16, tag="agi1")
                    nc.sync.dma_start(agi1[:], h1loc[:])
                    ago1 = BP.tile([H1, B], bf16, tag="ago1")
                    nc.gpsimd.collective_compute("AllGather", OP.bypass, replica_groups=RG,
                                                 ins=[agi1.opt()], outs=[ago1.opt()])
                    nc.sync.dma_start(h1T[:], ago1[:].rearrange("(k p) n -> p k n", p=128))

                # ---- g2 partial: Wh2@h2 + W2h1@h1n ----
                g2ps = PG2.tile([B, GS], f32, tag="g2")
                for k in range(KT1):
                    nc.tensor.matmul(g2ps[0:n, :], h2T[:, k, 0:n], wh2[:, k, :],
                                     start=(k == 0), stop=False)
                for k in range(KT1):
                    nc.tensor.matmul(g2ps[0:n, :], h1T[:, k, 0:n], w2h1[:, k, :],
                                     start=False, stop=False)

                # ---- attention: p = h1@Wha.T + bha ; tanh(Va + p) ; logit ----
                pps = PSM.tile([B, 512], f32, tag="small")
                for k in range(KT1):
                    nc.tensor.matmul(pps[:, 0:PH], h1T[:, k, :], wha[:, k, :],
                                     start=(k == 0), stop=(k == KT1 - 1))
                pbs = W.tile([B, PH], bf16, tag="pbs")
                nc.vector.tensor_tensor(out=pbs[:], in0=pps[:, 0:PH], in1=bhas[:], op=OP.add)
                tnh = W.tile([B, RP, PH], bf16, tag="tnh")
                vap = W.tile([B, RP, PH], bf16, tag="vap")
                nc.vector.tensor_tensor(out=vap[:], in0=Va[:],
                                        in1=pbs[:].unsqueeze(1).broadcast_to([B, RP, PH]), op=OP.add)
                nc.scalar.activation(tnh[:], vap[:], AT.Tanh)
                logit = W.tile([B, RP], f32, tag="logit")
                wprod = W.tile([B, RP, PH], bf16, tag="wprod")
                nc.vector.tensor_tensor(
                    out=wprod[:], in0=tnh[:],
                    in1=was[:].unsqueeze(1).broadcast_to([B, RP, PH]), op=OP.mult)
                nc.vector.tensor_reduce(logit[:], wprod[:], axis=mybir.AxisListType.X,
                                        op=OP.add)
                # transpose logits to [RP, B], apply pad mask, AllGather
                ltp = PSM.tile([RP, B], f32, tag="ltp")
                nc.tensor.transpose(ltp[:], logit[:], identf[:])
                agi2 = BP.tile([RP, B], f32, tag="agi2")
                lts = W.tile([RP, B], f32, tag="lts")
                nc.vector.tensor_scalar(out=lts[:], in0=ltp[:],
                                        scalar1=lmasks[:], scalar2=loffs[:],
                                        op0=OP.mult, op1=OP.add)
                lall = W.tile([RT, B], f32, tag="lall")
                if NO_CC:
                    nc.vector.memset(lall[:], 0.0)
                    nc.vector.tensor_copy(lall[0:RP, :], lts[:])
                else:
                    nc.sync.dma_start(agi2[:], lts[:])
                    ago2 = BP.tile([RT, B], f32, tag="ago2")
                    nc.gpsimd.collective_compute("AllGather", OP.bypass, replica_groups=RG,
                                                 ins=[agi2.opt()], outs=[ago2.opt()])
                    nc.sync.dma_start(lall[:], ago2[:])

                # ---- softmax over r (b on partitions) ----
                lps = PSM.tile([B, 512], f32, tag="small")
                nc.tensor.transpose(lps[:, 0:RT], lall[:], identf[0:RT, 0:RT])
                esb = W.tile([B, RT], bf16, tag="esb")
                ssum = W.tile([B, 1], f32, tag="ssum")
                nc.scalar.activation(esb[:], lps[:, 0:RT], AT.Exp, accum_out=ssum[:])
                sinv = W.tile([B, 1], f32, tag="sinv")
                nc.vector.reciprocal(sinv[:], ssum[:])
                attn = W.tile([B, RT], f32, tag="attn")
                nc.vector.tensor_scalar(out=attn[:], in0=esb[:], scalar1=sinv[:],
                                        scalar2=None, op0=OP.mult)
                atp = PSM.tile([RT, 132], f32, tag="ltp")
                nc.tensor.transpose(atp[:, 0:B], attn[:], identf[:])
                # scatter into block-diag lhsT (2 strided copies, stride 130)
                for cc in range(2):
                    nc.vector.tensor_copy(
                        atten_bd[64 * cc:64 * cc + RT,
                                 cc:cc + 130 * (NPAIR - 1) + 1:130],
                        atp[:][:, cc:cc + 2 * (NPAIR - 1) + 1:2])

                # ---- einsum via block-diag matmuls, accumulate into g2 psum ----
                npr_t = (n + 1) // 2
                for i in range(npr_t):
                    nc.tensor.matmul(g2ps[0:n, :],
                                     atten_bd[0:BDK, 128 * i:128 * i + n],
                                     VW2[0:BDK, i, :],
                                     start=False, stop=(i == npr_t - 1))

                # ---- LSTM2 cell ----
                gs2 = W.tile([B, GS], f32, tag="gs1")
                nc.vector.tensor_tensor(out=gs2[0:n, :], in0=g2ps[0:n, :],
                                        in1=b2s[0:n, :], op=OP.add)
                tio2 = W.tile([B, 3 * HS], f32, tag="tio1")
                nc.scalar.activation(tio2[0:n, :], gs2[0:n, 0:3 * HS], AT.Tanh, scale=0.5)
                tg2 = W.tile([B, HS], f32, tag="tg1")
                nc.scalar.activation(tg2[0:n, :], gs2[0:n, 3 * HS:], AT.Tanh)
                sg2 = W.tile([B, 3 * HS], f32, tag="sg1")
                nc.vector.tensor_scalar(out=sg2[0:n, :], in0=tio2[0:n, :],
                                        scalar1=0.5, scalar2=0.5, op0=OP.mult, op1=OP.add)
                aa2 = W.tile([B, HS], f32, tag="aa")
                nc.vector.tensor_tensor(out=aa2[0:n, :], in0=sg2[0:n, HS:2 * HS],
                                        in1=c2[0:n, :], op=OP.mult)
                bb2 = W.tile([B, HS], f32, tag="bb")
                nc.vector.tensor_tensor(out=bb2[0:n, :], in0=sg2[0:n, 0:HS],
                                        in1=tg2[0:n, :], op=OP.mult)
                nc.vector.tensor_tensor(out=c2[0:n, :], in0=aa2[0:n, :],
                                        in1=bb2[0:n, :], op=OP.add)
                tc2 = W.tile([B, HS], f32, tag="tc1")
                nc.scalar.activation(tc2[0:n, :], c2[0:n, :], AT.Tanh)
                nc.vector.tensor_tensor(out=h2n[0:n, :], in0=sg2[0:n, 2 * HS:],
                                        in1=tc2[0:n, :], op=OP.mult)

                # ---- AllGather h2T; store compacted history ----
                h2loc = W.tile([128, B], bf16, tag="h1loc")
                if NO_DMAT:
                    tp2 = PSM.tile([128, B], bf16, tag="ltp", name="tp2")
                    nc.tensor.transpose(tp2[:], h2n[:], ident[:])
                    nc.vector.tensor_copy(h2loc[:], tp2[:])
                else:
                    nc.sync.dma_start_transpose(h2loc[:], h2n[:])
                if NO_CC:
                    for kk in range(KT1):
                        nc.vector.tensor_copy(h2T[:, kk, :], h2loc[:])
                else:
                    agi3 = BP.tile([128, B], bf16, tag="agi3")
                    nc.sync.dma_start(agi3[:], h2loc[:])
                    ago3 = BP.tile([H2, B], bf16, tag="ago3")
                    nc.gpsimd.collective_compute("AllGather", OP.bypass, replica_groups=RG,
                                                 ins=[agi3.opt()], outs=[ago3.opt()])
                    nc.sync.dma_start(h2T[:], ago3[:].rearrange("(k p) n -> p k n", p=128))
                o0 = int(offs[t])
                nc.sync.dma_start(
                    h2a_d[:].rearrange("(k p) n -> p k n", p=128)[:, :, o0:o0 + n],
                    h2T[:, :, 0:n])

        # ================= VOCAB PROJECTION =================
        if PHASE in ("pre", "loop"):
            NMT = 0
        with (
            tc.tile_pool(name="wl_w", bufs=1) as WW,
            tc.tile_pool(name="wl_sb", bufs=4) as WS,
            tc.tile_pool(name="wl_ps", bufs=4, space="PSUM") as WP,
        ):
            wl = WW.tile([128, KT1, VS], bf16)
            nc.sync.dma_start(wl[:], WlT.ap().rearrange("(k p) n -> p k n", p=128))
            bls = WW.tile([B, VS], f32); nc.sync.dma_start(bls[:], blb[:])
            # column -> (t, b) map for output scatter
            segs = []  # per M-tile: list of (row0, t, b0, cnt)
            col_t = np.repeat(np.arange(Tsteps), np.array(n_t[:Tsteps], dtype=np.int64))
            col_b = np.concatenate([np.arange(n_t[t]) for t in range(Tsteps)]) \
                if (NCOMP and Tsteps) else np.zeros(0, np.int64)
            for m in range(NMT):
                lo, hi = m * 128, min(NCOMP, m * 128 + 128)
                rows = []
                j = lo
                while j < hi:
                    tt = int(col_t[j]); b0 = int(col_b[j])
                    cnt = 1
                    while j + cnt < hi and col_t[j + cnt] == tt:
                        cnt += 1
                    rows.append((j - lo, tt, b0, cnt))
                    j += cnt
                segs.append(rows)

            NVT = (VS + 511) // 512
            for m in range(NMT):
                lo, hi = m * 128, min(NCOMP, m * 128 + 128)
                mw = hi - lo
                h2t = WS.tile([128, KT1, 128], bf16, tag="h2t")
                nc.sync.dma_start(
                    h2t[:, :, 0:mw],
                    h2a_d[:].rearrange("(k p) n -> p k n", p=128)[:, :, lo:hi])
                for v in range(NVT):
                    v0, v1 = v * 512, min(VS, v * 512 + 512)
                    wps = WP.tile([128, 512], f32, tag="wps")
                    for k in range(KT1):
                        nc.tensor.matmul(wps[0:mw, 0:v1 - v0], h2t[:, k, 0:mw],
                                         wl[:, k, v0:v1],
                                         start=(k == 0), stop=(k == KT1 - 1))
                    ores = WS.tile([128, 512], f32, tag="ores")
                    nc.vector.tensor_tensor(out=ores[0:mw, 0:v1 - v0],
                                            in0=wps[0:mw, 0:v1 - v0],
                                            in1=bls[0:mw, v0:v1], op=OP.add)
                    for (r0, tt, b0, cnt) in segs[m]:
                        nc.sync.dma_start(
                            out.ap()[b0:b0 + cnt, tt, v0:v1],
                            ores[r0:r0 + cnt, 0:v1 - v0])

    nc.compile()
    return nc


def kernel(**inputs) -> np.ndarray:
    from concourse.bass_utils import run_bass_kernel_spmd

    in_maps, n_t, offs = _prep_inputs(inputs)
    Tsteps = int(os.environ.get("K_TSTEPS", T))
    # WvaT is declared inside _build via dram_tensor; provide the data
    WvaT = np.ascontiguousarray(inputs["Wva"].T).astype(BF16)
    for m in in_maps:
        m["WvaT"] = WvaT

    key = (tuple(n_t), Tsteps)
    if key not in _cache:
        _cache[key] = _build(n_t, offs, Tsteps)
    nc = _cache[key]

    global LAST_NC, LAST_IN_MAPS
    LAST_NC, LAST_IN_MAPS = nc, in_maps
    res = run_bass_kernel_spmd(nc, in_maps, core_ids=list(range(NC_)))
    outs = [res.results[c]["out"] for c in range(NC_)]
    full = np.concatenate(outs, axis=2).astype(np.float32)
    return full


if __name__ == "__main__":
    # quick shape sanity
    print("kernel module OK")

